# revision 45
# baseline (speedup 1.0000x reference)
"""Trainium2 Bass kernel for AverageSpanExtractor (segment mean over spans).

Math note: the reference's masked softmax over all-ones logits reduces
exactly to a mean over the span tokens [start, end):
    out[b, n, :] = mean(sequence_tensor[b, start:end, :]).

Strategy (8 cores, batch-parallel — one batch element per core):
sorted-span segment matmuls with host-marshalled operands. Measured
constraints that shaped this: indexed fetches (SWDGE gather /
ap_gather / indirect_copy) cost >=20us for the ~3k random rows needed;
the PE is throttled to ~1.2GHz with ~150-300ns fixed cost per
instruction; DVE tensor ops run ~1.2ns/elem, making on-device
indicator construction (~770K elems x3 ops) a ~22us serial chain. So
everything data-independent moves to the host:

  1. HOST sorts each batch's spans by start; a 128-span chunk of the
     sorted order covers 5-7 of the 32 128-token blocks (window bounds
     unioned across cores; nc built per span-structure and cached).
  2. HOST materializes the binary token-major indicator MT[t, w, i]
     (f16, exact) for every window w and ships it (1.5 MB), along with
     the sequence pre-cast to f16 (2.1 MB) and 1/w (f32).
  3. DEVICE: per chunk j, K_j accumulating matmuls
        out_j += MT_w.T @ x_block      (f16, f32 PSUM)
     with two chunks' chains interleaved to hide PSUM group
     transitions. Scale rows by 1/w on the DVE during PSUM->SBUF,
     store contiguous (sorted order); HOST unpermutes rows.

Precision: indicator exact; x f16 (2^-11) => ~2e-4 global rel err.
"""

import numpy as np

B, S, D = 8, 4096, 256
N_SPANS = 1024
P = 128
NBLK = S // P
JG = N_SPANS // P      # 8 span chunks of 128

_cache = {"key": None}


def _plan_windows(si):
    """Sorted-span chunk block windows, unioned across cores."""
    perms = np.empty((B, N_SPANS), dtype=np.int64)
    ss = np.empty((B, N_SPANS), dtype=np.int64)
    ee = np.empty((B, N_SPANS), dtype=np.int64)
    for b in range(B):
        perm = np.argsort(si[b, :, 0], kind="stable")
        perms[b] = perm
        ss[b] = si[b, perm, 0]
        ee[b] = si[b, perm, 1]
    windows = []
    for j in range(JG):
        b0 = NBLK
        b1 = 0
        for b in range(B):
            cs = ss[b, j * P : (j + 1) * P]
            ce = ee[b, j * P : (j + 1) * P]
            b0 = min(b0, int(cs.min()) >> 7)
            b1 = max(b1, (int(ce.max()) - 1) >> 7)
        windows.append((b0, b1 - b0 + 1))
    return perms, windows, ss, ee


def build_nc(windows):
    import concourse.bacc as bacc
    import concourse.mybir as mybir
    from concourse.tile import TileContext

    f32 = mybir.dt.float32
    f16 = mybir.dt.float16

    wbase = []
    w0 = 0
    for j in range(JG):
        wbase.append(w0)
        w0 += windows[j][1]
    NW = w0

    nc = bacc.Bacc(None, target_bir_lowering=False, debug=False, num_devices=B)
    # partition-major: seqh[p, b*D:d] = seq[128b+p, d] — contiguous 2KB/
    # partition descriptors instead of 512B (which pay 2x DMA latency)
    seqh = nc.declare_dram_parameter("seqh", [P, NBLK * D], f16, isOutput=False)
    mtd = nc.declare_dram_parameter("mtd", [P, NW * P], f16, isOutput=False)
    wrec = nc.declare_dram_parameter("wrec", [P, JG], f32, isOutput=False)
    out = nc.declare_dram_parameter("out", [N_SPANS, D], f16, isOutput=True)

    with TileContext(nc) as tc:
        with (
            tc.tile_pool(name="x", bufs=1) as x_pool,
            tc.tile_pool(name="ps", bufs=2, space="PSUM") as ps_pool,
            tc.tile_pool(name="misc", bufs=1) as misc_pool,
        ):
            WR = misc_pool.tile([P, JG], f32)
            MTbig = misc_pool.tile([P, NW, P], f16)

            def load_mt_w(w0, w1):
                # MT rides the ACT ring so seq owns the SP ring
                nc.scalar.dma_start(
                    out=MTbig[:, w0:w1, :], in_=mtd[:, w0 * P : w1 * P]
                )

            def load_mt(j0, j1):
                load_mt_w(wbase[j0], wbase[j1 - 1] + windows[j1 - 1][1])

            # small groups early and mid-stream so the PE never starves
            GSIZES = [2, 2, 2, 2] + [4] * ((NBLK - 8) // 4)
            NG = len(GSIZES)
            GOFF = [sum(GSIZES[:g]) for g in range(NG)]
            bigxs = [None] * NG

            def emit_load(g, eng=None):
                gb = GSIZES[g]
                bigx = x_pool.tile([P, gb * D], f16, name=f"bigx{g}")
                (eng or nc.sync).dma_start(
                    out=bigx[:],
                    in_=seqh[:, GOFF[g] * D : (GOFF[g] + gb) * D],
                )
                bigxs[g] = bigx

            # first two windows unblock the PE's first matmuls ASAP;
            # WR is only needed by the first finish (~15us), so it goes last.
            # x groups alternate rings so early blocks arrive at 2x rate.
            load_mt_w(0, 2)
            emit_load(1, eng=nc.scalar)
            load_mt_w(2, wbase[1] + windows[1][1])
            emit_load(3, eng=nc.scalar)
            load_mt(2, 4)
            emit_load(5, eng=nc.scalar)
            load_mt(4, 6)
            load_mt(6, 8)
            nc.scalar.dma_start(out=WR[:], in_=wrec[:])
            for g in (0, 2, 4, 6, 7, 8, 9):
                emit_load(g)

            def blk_rhs(blk):
                g = next(
                    g for g in range(NG) if GOFF[g] <= blk < GOFF[g] + GSIZES[g]
                )
                lo = (blk - GOFF[g]) * D
                return bigxs[g][:, lo : lo + D]

            def emit_mm_pair(*js):
                # interleave chunks' accumulation chains so PSUM group
                # transitions of one hide under the others
                chains = []
                for j in js:
                    if j is None or j >= JG:
                        continue
                    b0, kj = windows[j]
                    ps = ps_pool.tile([P, D], f32, name=f"ps{j % 4}")
                    chains.append((j, b0, kj, ps))
                def finish(j, ps):
                    rj = misc_pool.tile([P, D], f16, name=f"rj{j}")
                    if j % 2 == 0:
                        nc.vector.tensor_scalar_mul(
                            out=rj[:], in0=ps[:], scalar1=WR[:, j : j + 1]
                        )
                    else:
                        nc.scalar.activation(
                            out=rj[:], in_=ps[:],
                            func=mybir.ActivationFunctionType.Copy,
                            scale=WR[:, j : j + 1],
                        )
                    oj = out[:].rearrange("(c p) d -> p c d", p=P)[:, j, :]
                    seng = nc.scalar if j % 2 == 0 else nc.sync
                    seng.dma_start(out=oj, in_=rj[:])

                maxk = max(c[2] for c in chains)
                for bb in range(maxk):
                    for j, b0, kj, ps in chains:
                        if bb < kj:
                            nc.tensor.matmul(
                                out=ps[:],
                                lhsT=MTbig[:, wbase[j] + bb, :],
                                rhs=blk_rhs(b0 + bb),
                                start=(bb == 0), stop=(bb == kj - 1),
                            )
                        if bb == kj - 1:
                            finish(j, ps)

            emit_mm_pair(0, 1)
            emit_mm_pair(2, 3)
            emit_mm_pair(4, 5, 6, 7)
    nc.finalize()
    return nc


def _make_in_maps(sequence_tensor, si, perms, windows, ss, ee):
    seqf = np.asarray(sequence_tensor).astype(np.float16)
    seqh = np.ascontiguousarray(
        seqf.reshape(B, NBLK, P, D).transpose(0, 2, 1, 3).reshape(B, P, NBLK * D)
    )
    NW = sum(k for _, k in windows)
    tok = np.arange(P, dtype=np.int64)[:, None]  # [128 t, 1]
    in_maps = []
    for b in range(B):
        mt = np.zeros((P, NW, P), dtype=np.float16)
        w = 0
        for j in range(JG):
            b0, kj = windows[j]
            cs = ss[b, j * P : (j + 1) * P][None, :]  # [1, 128 spans]
            ce = ee[b, j * P : (j + 1) * P][None, :]
            for bb in range(kj):
                tg = tok + 128 * (b0 + bb)
                mt[:, w, :] = ((tg >= cs) & (tg < ce)).astype(np.float16)
                w += 1
        wr = (
            1.0 / (ee[b] - ss[b]).astype(np.float32)
        ).reshape(JG, P).T.copy()
        in_maps.append(
            {
                "seqh": seqh[b],
                "mtd": np.ascontiguousarray(mt.reshape(P, NW * P)),
                "wrec": wr,
            }
        )
    return in_maps


def kernel(sequence_tensor, span_indices):
    from concourse.bass_utils import run_bass_kernel_spmd

    si = np.asarray(span_indices)
    assert si.shape == (B, N_SPANS, 2)
    key = si.tobytes()
    if _cache["key"] != key:
        perms, windows, ss, ee = _plan_windows(si)
        _cache.update(
            key=key, nc=build_nc(windows),
            plan=(perms, windows, ss, ee),
        )
    perms, windows, ss, ee = _cache["plan"]
    in_maps = _make_in_maps(sequence_tensor, si, perms, windows, ss, ee)
    res = run_bass_kernel_spmd(_cache["nc"], in_maps, list(range(B)))
    full = np.empty((B, N_SPANS, D), dtype=np.float32)
    for b in range(B):
        full[b, perms[b], :] = res.results[b]["out"].astype(np.float32)
    return full


# revision 46
# speedup vs baseline: 1.0483x; 1.0483x over previous
"""Trainium2 Bass kernel for AverageSpanExtractor (segment mean over spans).

Math note: the reference's masked softmax over all-ones logits reduces
exactly to a mean over the span tokens [start, end):
    out[b, n, :] = mean(sequence_tensor[b, start:end, :]).

Strategy (8 cores, batch-parallel — one batch element per core):
sorted-span segment matmuls with host-marshalled operands. Measured
constraints that shaped this: indexed fetches (SWDGE gather /
ap_gather / indirect_copy) cost >=20us for the ~3k random rows needed;
the PE is throttled to ~1.2GHz with ~150-300ns fixed cost per
instruction; DVE tensor ops run ~1.2ns/elem, making on-device
indicator construction (~770K elems x3 ops) a ~22us serial chain. So
everything data-independent moves to the host:

  1. HOST sorts each batch's spans by start; a 128-span chunk of the
     sorted order covers 5-7 of the 32 128-token blocks (window bounds
     unioned across cores; nc built per span-structure and cached).
  2. HOST materializes the binary token-major indicator MT[t, w, i]
     (f16, exact) for every window w and ships it (1.5 MB), along with
     the sequence pre-cast to f16 (2.1 MB) and 1/w (f32).
  3. DEVICE: per chunk j, K_j accumulating matmuls
        out_j += MT_w.T @ x_block      (f16, f32 PSUM)
     with two chunks' chains interleaved to hide PSUM group
     transitions. Scale rows by 1/w on the DVE during PSUM->SBUF,
     store contiguous (sorted order); HOST unpermutes rows.

Precision: indicator exact; x f16 (2^-11) => ~2e-4 global rel err.
"""

import numpy as np

B, S, D = 8, 4096, 256
N_SPANS = 1024
P = 128
NBLK = S // P
JG = N_SPANS // P      # 8 span chunks of 128

_cache = {"key": None}


def _plan_windows(si):
    """Sorted-span chunk block windows, unioned across cores."""
    perms = np.empty((B, N_SPANS), dtype=np.int64)
    ss = np.empty((B, N_SPANS), dtype=np.int64)
    ee = np.empty((B, N_SPANS), dtype=np.int64)
    for b in range(B):
        perm = np.argsort(si[b, :, 0], kind="stable")
        perms[b] = perm
        ss[b] = si[b, perm, 0]
        ee[b] = si[b, perm, 1]
    windows = []
    for j in range(JG):
        b0 = NBLK
        b1 = 0
        for b in range(B):
            cs = ss[b, j * P : (j + 1) * P]
            ce = ee[b, j * P : (j + 1) * P]
            b0 = min(b0, int(cs.min()) >> 7)
            b1 = max(b1, (int(ce.max()) - 1) >> 7)
        windows.append((b0, b1 - b0 + 1))
    return perms, windows, ss, ee


def build_nc(windows):
    import concourse.bacc as bacc
    import concourse.mybir as mybir
    from concourse.tile import TileContext

    f32 = mybir.dt.float32
    f16 = mybir.dt.float16

    wbase = []
    w0 = 0
    for j in range(JG):
        wbase.append(w0)
        w0 += windows[j][1]
    NW = w0

    nc = bacc.Bacc(None, target_bir_lowering=False, debug=False, num_devices=B)
    # partition-major: seqh[p, b*D:d] = seq[128b+p, d] — contiguous 2KB/
    # partition descriptors instead of 512B (which pay 2x DMA latency)
    seqh = nc.declare_dram_parameter("seqh", [P, NBLK * D], f16, isOutput=False)
    mtd = nc.declare_dram_parameter("mtd", [P, NW * P], f16, isOutput=False)
    wrec = nc.declare_dram_parameter("wrec", [P, JG], f32, isOutput=False)
    out = nc.declare_dram_parameter("out", [N_SPANS, D], f16, isOutput=True)

    with TileContext(nc) as tc:
        with (
            tc.tile_pool(name="x", bufs=1) as x_pool,
            tc.tile_pool(name="ps", bufs=2, space="PSUM") as ps_pool,
            tc.tile_pool(name="misc", bufs=1) as misc_pool,
        ):
            WR = misc_pool.tile([P, JG], f32)
            MTbig = misc_pool.tile([P, NW, P], f16)

            def load_mt_w(w0, w1):
                # MT rides the ACT ring so seq owns the SP ring
                nc.scalar.dma_start(
                    out=MTbig[:, w0:w1, :], in_=mtd[:, w0 * P : w1 * P]
                )

            def load_mt(j0, j1):
                load_mt_w(wbase[j0], wbase[j1 - 1] + windows[j1 - 1][1])

            # small groups early and mid-stream so the PE never starves
            GSIZES = [2, 2, 2, 2] + [4] * ((NBLK - 8) // 4)
            NG = len(GSIZES)
            GOFF = [sum(GSIZES[:g]) for g in range(NG)]
            bigxs = [None] * NG

            def emit_load(g, eng=None):
                gb = GSIZES[g]
                bigx = x_pool.tile([P, gb * D], f16, name=f"bigx{g}")
                (eng or nc.sync).dma_start(
                    out=bigx[:],
                    in_=seqh[:, GOFF[g] * D : (GOFF[g] + gb) * D],
                )
                bigxs[g] = bigx

            # first two windows unblock the PE's first matmuls ASAP;
            # WR is only needed by the first finish (~15us), so it goes last
            load_mt_w(0, 2)
            load_mt_w(2, wbase[1] + windows[1][1])
            load_mt(2, 4)
            load_mt(4, 6)
            load_mt(6, 8)
            nc.scalar.dma_start(out=WR[:], in_=wrec[:])
            for g in range(NG):
                emit_load(g)

            def blk_rhs(blk):
                g = next(
                    g for g in range(NG) if GOFF[g] <= blk < GOFF[g] + GSIZES[g]
                )
                lo = (blk - GOFF[g]) * D
                return bigxs[g][:, lo : lo + D]

            def emit_mm_pair(*js):
                # interleave chunks' accumulation chains so PSUM group
                # transitions of one hide under the others
                chains = []
                for j in js:
                    if j is None or j >= JG:
                        continue
                    b0, kj = windows[j]
                    ps = ps_pool.tile([P, D], f32, name=f"ps{j % 4}")
                    chains.append((j, b0, kj, ps))
                def finish(j, ps):
                    rj = misc_pool.tile([P, D], f16, name=f"rj{j}")
                    if j % 2 == 0:
                        nc.vector.tensor_scalar_mul(
                            out=rj[:], in0=ps[:], scalar1=WR[:, j : j + 1]
                        )
                    else:
                        nc.scalar.activation(
                            out=rj[:], in_=ps[:],
                            func=mybir.ActivationFunctionType.Copy,
                            scale=WR[:, j : j + 1],
                        )
                    oj = out[:].rearrange("(c p) d -> p c d", p=P)[:, j, :]
                    seng = nc.scalar if j % 2 == 0 else nc.sync
                    seng.dma_start(out=oj, in_=rj[:])

                maxk = max(c[2] for c in chains)
                for bb in range(maxk):
                    for j, b0, kj, ps in chains:
                        if bb < kj:
                            nc.tensor.matmul(
                                out=ps[:],
                                lhsT=MTbig[:, wbase[j] + bb, :],
                                rhs=blk_rhs(b0 + bb),
                                start=(bb == 0), stop=(bb == kj - 1),
                            )
                        if bb == kj - 1:
                            finish(j, ps)

            emit_mm_pair(0, 1)
            emit_mm_pair(2, 3)
            emit_mm_pair(4, 5, 6, 7)
    nc.finalize()
    return nc


def _make_in_maps(sequence_tensor, si, perms, windows, ss, ee):
    seqf = np.asarray(sequence_tensor).astype(np.float16)
    seqh = np.ascontiguousarray(
        seqf.reshape(B, NBLK, P, D).transpose(0, 2, 1, 3).reshape(B, P, NBLK * D)
    )
    NW = sum(k for _, k in windows)
    tok = np.arange(P, dtype=np.int64)[:, None]  # [128 t, 1]
    in_maps = []
    for b in range(B):
        mt = np.zeros((P, NW, P), dtype=np.float16)
        w = 0
        for j in range(JG):
            b0, kj = windows[j]
            cs = ss[b, j * P : (j + 1) * P][None, :]  # [1, 128 spans]
            ce = ee[b, j * P : (j + 1) * P][None, :]
            for bb in range(kj):
                tg = tok + 128 * (b0 + bb)
                mt[:, w, :] = ((tg >= cs) & (tg < ce)).astype(np.float16)
                w += 1
        wr = (
            1.0 / (ee[b] - ss[b]).astype(np.float32)
        ).reshape(JG, P).T.copy()
        in_maps.append(
            {
                "seqh": seqh[b],
                "mtd": np.ascontiguousarray(mt.reshape(P, NW * P)),
                "wrec": wr,
            }
        )
    return in_maps


def kernel(sequence_tensor, span_indices):
    from concourse.bass_utils import run_bass_kernel_spmd

    si = np.asarray(span_indices)
    assert si.shape == (B, N_SPANS, 2)
    key = si.tobytes()
    if _cache["key"] != key:
        perms, windows, ss, ee = _plan_windows(si)
        _cache.update(
            key=key, nc=build_nc(windows),
            plan=(perms, windows, ss, ee),
        )
    perms, windows, ss, ee = _cache["plan"]
    in_maps = _make_in_maps(sequence_tensor, si, perms, windows, ss, ee)
    res = run_bass_kernel_spmd(_cache["nc"], in_maps, list(range(B)))
    full = np.empty((B, N_SPANS, D), dtype=np.float32)
    for b in range(B):
        full[b, perms[b], :] = res.results[b]["out"].astype(np.float32)
    return full


# revision 47
# speedup vs baseline: 1.2204x; 1.1641x over previous
"""Trainium2 Bass kernel for AverageSpanExtractor (segment mean over spans).

Math note: the reference's masked softmax over all-ones logits reduces
exactly to a mean over the span tokens [start, end):
    out[b, n, :] = mean(sequence_tensor[b, start:end, :]).

Strategy (8 cores, batch-parallel — one batch element per core):
sorted-span segment matmuls with host-marshalled operands. Measured
constraints that shaped this: indexed fetches (SWDGE gather /
ap_gather / indirect_copy) cost >=20us for the ~3k random rows needed;
the PE is throttled to ~1.2GHz with ~150-300ns fixed cost per
instruction; DVE tensor ops run ~1.2ns/elem, making on-device
indicator construction (~770K elems x3 ops) a ~22us serial chain. So
everything data-independent moves to the host:

  1. HOST sorts each batch's spans by start; a 128-span chunk of the
     sorted order covers 5-7 of the 32 128-token blocks (window bounds
     unioned across cores; nc built per span-structure and cached).
  2. HOST materializes the binary token-major indicator MT[t, w, i]
     (f16, exact) for every window w and ships it (1.5 MB), along with
     the sequence pre-cast to f16 (2.1 MB) and 1/w (f32).
  3. DEVICE: per chunk j, K_j accumulating matmuls
        out_j += MT_w.T @ x_block      (f16, f32 PSUM)
     with two chunks' chains interleaved to hide PSUM group
     transitions. Scale rows by 1/w on the DVE during PSUM->SBUF,
     store contiguous (sorted order); HOST unpermutes rows.

Precision: indicator exact; x f16 (2^-11) => ~2e-4 global rel err.
"""

import numpy as np

B, S, D = 8, 4096, 256
N_SPANS = 1024
P = 128
NBLK = S // P
JG = N_SPANS // P      # 8 span chunks of 128

_cache = {"key": None}


def _to_f8(a):
    """Encode 0.0/1.0 as float8e4m3 bytes (0x00 / 0x38) for the DMA."""
    import concourse.mybir as mybir
    npdt = mybir.dt.np(mybir.dt.float8e4)
    return np.ascontiguousarray(a.astype(np.float32)).astype(npdt)


def _plan_windows(si):
    """Sorted-span chunk block windows, unioned across cores."""
    perms = np.empty((B, N_SPANS), dtype=np.int64)
    ss = np.empty((B, N_SPANS), dtype=np.int64)
    ee = np.empty((B, N_SPANS), dtype=np.int64)
    for b in range(B):
        perm = np.argsort(si[b, :, 0], kind="stable")
        perms[b] = perm
        ss[b] = si[b, perm, 0]
        ee[b] = si[b, perm, 1]
    windows = []
    for j in range(JG):
        b0 = NBLK
        b1 = 0
        for b in range(B):
            cs = ss[b, j * P : (j + 1) * P]
            ce = ee[b, j * P : (j + 1) * P]
            b0 = min(b0, int(cs.min()) >> 7)
            b1 = max(b1, (int(ce.max()) - 1) >> 7)
        windows.append((b0, b1 - b0 + 1))
    return perms, windows, ss, ee


def build_nc(windows):
    import concourse.bacc as bacc
    import concourse.mybir as mybir
    from concourse.tile import TileContext

    f32 = mybir.dt.float32
    f16 = mybir.dt.float16

    wbase = []
    w0 = 0
    for j in range(JG):
        wbase.append(w0)
        w0 += windows[j][1]
    NW = w0

    nc = bacc.Bacc(None, target_bir_lowering=False, debug=False, num_devices=B)
    # partition-major: seqh[p, b*D:d] = seq[128b+p, d] — contiguous 2KB/
    # partition descriptors instead of 512B (which pay 2x DMA latency)
    seqh = nc.declare_dram_parameter("seqh", [P, NBLK * D], f16, isOutput=False)
    f8 = mybir.dt.float8e4
    mtd = nc.declare_dram_parameter("mtd", [P, NW * P], f8, isOutput=False)
    wrec = nc.declare_dram_parameter("wrec", [P, JG], f32, isOutput=False)
    out = nc.declare_dram_parameter("out", [N_SPANS, D], f16, isOutput=True)

    with TileContext(nc) as tc:
        with (
            tc.tile_pool(name="x", bufs=1) as x_pool,
            tc.tile_pool(name="ps", bufs=2, space="PSUM") as ps_pool,
            tc.tile_pool(name="misc", bufs=1) as misc_pool,
        ):
            WR = misc_pool.tile([P, JG], f32)
            MTbig = misc_pool.tile([P, NW, P], f8)

            def load_mt_w(w0, w1):
                # MT rides the ACT ring so seq owns the SP ring
                nc.scalar.dma_start(
                    out=MTbig[:, w0:w1, :], in_=mtd[:, w0 * P : w1 * P]
                )

            def load_mt(j0, j1):
                load_mt_w(wbase[j0], wbase[j1 - 1] + windows[j1 - 1][1])

            # small groups early and mid-stream so the PE never starves
            GSIZES = [2, 2, 2, 2] + [4] * ((NBLK - 8) // 4)
            NG = len(GSIZES)
            GOFF = [sum(GSIZES[:g]) for g in range(NG)]
            bigxs = [None] * NG

            def emit_load(g, eng=None):
                gb = GSIZES[g]
                bigx = x_pool.tile([P, gb * D], f16, name=f"bigx{g}")
                (eng or nc.sync).dma_start(
                    out=bigx[:],
                    in_=seqh[:, GOFF[g] * D : (GOFF[g] + gb) * D],
                )
                bigxs[g] = bigx

            # first two windows unblock the PE's first matmuls ASAP;
            # WR is only needed by the first finish (~15us), so it goes last
            load_mt_w(0, 2)
            load_mt_w(2, wbase[1] + windows[1][1])
            load_mt(2, 4)
            load_mt(4, 6)
            load_mt(6, 8)
            nc.scalar.dma_start(out=WR[:], in_=wrec[:])
            for g in range(NG):
                emit_load(g)

            def blk_rhs(blk):
                g = next(
                    g for g in range(NG) if GOFF[g] <= blk < GOFF[g] + GSIZES[g]
                )
                lo = (blk - GOFF[g]) * D
                return bigxs[g][:, lo : lo + D]

            def emit_mm_pair(*js):
                # interleave chunks' accumulation chains so PSUM group
                # transitions of one hide under the others
                chains = []
                for j in js:
                    if j is None or j >= JG:
                        continue
                    b0, kj = windows[j]
                    ps = ps_pool.tile([P, D], f32, name=f"ps{j % 4}")
                    chains.append((j, b0, kj, ps))
                def finish(j, ps):
                    rj = misc_pool.tile([P, D], f16, name=f"rj{j}")
                    if j % 2 == 0:
                        nc.vector.tensor_scalar_mul(
                            out=rj[:], in0=ps[:], scalar1=WR[:, j : j + 1]
                        )
                    else:
                        nc.scalar.activation(
                            out=rj[:], in_=ps[:],
                            func=mybir.ActivationFunctionType.Copy,
                            scale=WR[:, j : j + 1],
                        )
                    oj = out[:].rearrange("(c p) d -> p c d", p=P)[:, j, :]
                    seng = nc.scalar if j % 2 == 0 else nc.sync
                    seng.dma_start(out=oj, in_=rj[:])

                maxk = max(c[2] for c in chains)
                for bb in range(maxk):
                    for j, b0, kj, ps in chains:
                        if bb < kj:
                            nc.tensor.matmul(
                                out=ps[:],
                                lhsT=MTbig[:, wbase[j] + bb, :],
                                rhs=blk_rhs(b0 + bb),
                                start=(bb == 0), stop=(bb == kj - 1),
                            )
                        if bb == kj - 1:
                            finish(j, ps)

            emit_mm_pair(0, 1)
            emit_mm_pair(2, 3)
            emit_mm_pair(4, 5, 6, 7)
    nc.finalize()
    return nc


def _make_in_maps(sequence_tensor, si, perms, windows, ss, ee):
    seqf = np.asarray(sequence_tensor).astype(np.float16)
    seqh = np.ascontiguousarray(
        seqf.reshape(B, NBLK, P, D).transpose(0, 2, 1, 3).reshape(B, P, NBLK * D)
    )
    NW = sum(k for _, k in windows)
    tok = np.arange(P, dtype=np.int64)[:, None]  # [128 t, 1]
    in_maps = []
    for b in range(B):
        mt = np.zeros((P, NW, P), dtype=np.float16)  # cast to f8 below
        w = 0
        for j in range(JG):
            b0, kj = windows[j]
            cs = ss[b, j * P : (j + 1) * P][None, :]  # [1, 128 spans]
            ce = ee[b, j * P : (j + 1) * P][None, :]
            for bb in range(kj):
                tg = tok + 128 * (b0 + bb)
                mt[:, w, :] = ((tg >= cs) & (tg < ce)).astype(np.float16)
                w += 1
        wr = (
            1.0 / (ee[b] - ss[b]).astype(np.float32)
        ).reshape(JG, P).T.copy()
        in_maps.append(
            {
                "seqh": seqh[b],
                "mtd": _to_f8(mt.reshape(P, NW * P)),
                "wrec": wr,
            }
        )
    return in_maps


def kernel(sequence_tensor, span_indices):
    from concourse.bass_utils import run_bass_kernel_spmd

    si = np.asarray(span_indices)
    assert si.shape == (B, N_SPANS, 2)
    key = si.tobytes()
    if _cache["key"] != key:
        perms, windows, ss, ee = _plan_windows(si)
        _cache.update(
            key=key, nc=build_nc(windows),
            plan=(perms, windows, ss, ee),
        )
    perms, windows, ss, ee = _cache["plan"]
    in_maps = _make_in_maps(sequence_tensor, si, perms, windows, ss, ee)
    res = run_bass_kernel_spmd(_cache["nc"], in_maps, list(range(B)))
    full = np.empty((B, N_SPANS, D), dtype=np.float32)
    for b in range(B):
        full[b, perms[b], :] = res.results[b]["out"].astype(np.float32)
    return full
